# revision 4
# baseline (speedup 1.0000x reference)
"""Trainium2 Bass kernel for nn_DeChunkLayer.

Computation (per batch row):
  1. gate[c]: boundary-sorted clipped probabilities (host, tiny).
  2. EMA linear recurrence over chunks: h_c = (1-g_c) h_{c-1} + g_c x_c,
     computed on-device as a blocked lower-triangular matmul scan:
     per 128-chunk block t:  ema_t = L_t @ X_t + cp_t (x) h_prev
     where L_t[i,j] = g_j * prod_{k=j+1..i}(1-g_k), cp_t[i] = prod_{k<=i}(1-g_k)
     (host-computed in f64 via log-space cumsums).
  3. Dechunk: out[s] = ema[cid[s]] done as one-hot selection matmuls
     per 128-token block; selection matrices built on-device with
     is_equal against the replicated chunk-id row.

Sharding: pure data parallel, one batch row per NeuronCore (B=8, 8 cores).

Only chunk blocks that are actually referenced by tokens are processed
(NBLK = ceil(max_used_chunks/128)), all cores share one SPMD program whose
dechunk windows are the union of the per-core windows.
"""

import math

import numpy as np

import concourse.bass as bass
import concourse.mybir as mybir
from concourse import tile
from concourse.bass_utils import run_bass_kernel_spmd

B, SEQ, MAXC, DIM = 8, 4096, 2048, 1024
BLK = 128
NCORES = 8
NTB = SEQ // BLK  # 32 token blocks
F32 = mybir.dt.float32


def _preprocess(chunk_states, boundary_mask, boundary_prob):
    """Host-side index/gate math. Returns (in_maps, NBLK, windows)."""
    chunk_states = np.asarray(chunk_states, dtype=np.float32)
    boundary_mask = np.asarray(boundary_mask)
    boundary_prob = np.asarray(boundary_prob, dtype=np.float32)

    p_full = np.clip(boundary_prob[..., -1], np.float32(1e-4), np.float32(1.0 - 1e-4))
    token_idx = np.arange(SEQ)[None, :] + (~boundary_mask).astype(np.int32) * SEQ
    order = np.argsort(token_idx, axis=1, kind="stable")
    gate = np.take_along_axis(p_full, order[:, :MAXC], axis=1)  # [B, C]

    cid = np.cumsum(boundary_mask.astype(np.int32), axis=1) - 1  # [B, S]
    cid = np.clip(cid, 0, MAXC - 1)
    n_used = int(cid.max()) + 1
    NBLK = max(1, math.ceil(n_used / BLK))
    CU = NBLK * BLK

    # Per-block L^T and cp, in f64 log space (underflow -> 0 is fine).
    g = gate[:, :CU].astype(np.float64)
    a = 1.0 - g
    ls = np.cumsum(np.log(a).reshape(B, NBLK, BLK), axis=2)  # [B, NBLK, BLK]
    ii = np.arange(BLK)[:, None]
    jj = np.arange(BLK)[None, :]
    # L[b,t,i,j] = g[j] * exp(ls[i]-ls[j]) for i>=j
    Lf = np.where(
        ii[None, None] >= jj[None, None],
        np.exp(ls[:, :, :, None] - ls[:, :, None, :])
        * g.reshape(B, NBLK, 1, BLK),
        0.0,
    ).astype(np.float32)  # [B, NBLK, i, j]
    # ema rows are stored partition-REVERSED (chunk i of a block sits on
    # partition 127-i) so the block-carry row (last chunk) is partition 0,
    # which compute engines may legally address ([1,*] APs need base 0).
    Lf = Lf[:, :, ::-1, :]
    # lhsT layout: LT_sb[j, t*BLK+m] = L[t, rev(m), j]
    LT_sb = np.ascontiguousarray(
        Lf.transpose(0, 3, 1, 2).reshape(B, BLK, NBLK * BLK)
    )
    cp = np.exp(ls).astype(np.float32)[:, :, ::-1]  # [B, NBLK, BLK] reversed
    cp_sb = np.ascontiguousarray(cp.reshape(B, 1, NBLK * BLK))

    # dechunk union windows per token block
    cidr = cid.reshape(B, NTB, BLK)
    t0 = cidr[:, :, 0] // BLK  # [B, NTB]
    t1 = cidr[:, :, -1] // BLK
    lo = t0.min(axis=0)  # [NTB]
    hi = t1.max(axis=0)
    windows = [list(range(int(lo[tb]), int(hi[tb]) + 1)) for tb in range(NTB)]
    ncols = sum(len(w) for w in windows)
    jvec = np.empty((BLK, ncols), dtype=np.float32)
    col = 0
    for tb in range(NTB):
        for t in windows[tb]:
            jvec[:, col] = t * BLK + (BLK - 1 - np.arange(BLK))
            col += 1

    in_maps = []
    for b in range(B):
        in_maps.append(
            {
                "x": np.ascontiguousarray(chunk_states[b, :CU]),
                "lt": LT_sb[b],
                "cp": cp_sb[b],
                "cidb": np.ascontiguousarray(
                    np.broadcast_to(
                        cid[b].astype(np.float32)[None, :], (BLK, SEQ)
                    )
                ),
                "jvec": jvec,
            }
        )
    return in_maps, NBLK, windows


def _build_nc(NBLK, windows):
    ncols = sum(len(w) for w in windows)
    nc = bass.Bass()
    x = nc.dram_tensor("x", [NBLK * BLK, DIM], F32, kind="ExternalInput")
    lt = nc.dram_tensor("lt", [BLK, NBLK * BLK], F32, kind="ExternalInput")
    cp = nc.dram_tensor("cp", [1, NBLK * BLK], F32, kind="ExternalInput")
    cidb = nc.dram_tensor("cidb", [BLK, SEQ], F32, kind="ExternalInput")
    jvec = nc.dram_tensor("jvec", [BLK, ncols], F32, kind="ExternalInput")
    out = nc.dram_tensor("out", [SEQ, DIM], F32, kind="ExternalOutput")

    with tile.TileContext(nc) as tc:
        with (
            tc.tile_pool(name="const", bufs=1) as const_pool,
            tc.tile_pool(name="xp", bufs=4) as xpool,
            tc.tile_pool(name="hp", bufs=2) as hpool,
            tc.tile_pool(name="selp", bufs=8) as selpool,
            tc.tile_pool(name="outp", bufs=4) as outpool,
            tc.tile_pool(name="ps_scan", bufs=2, space="PSUM") as ps_scan,
            tc.tile_pool(name="ps_out", bufs=2, space="PSUM") as ps_out,
        ):
            lt_sb = const_pool.tile([BLK, NBLK * BLK], F32, tag="lt")
            nc.sync.dma_start(lt_sb[:], lt[:])
            cp_sb = const_pool.tile([1, NBLK * BLK], F32, tag="cp")
            nc.sync.dma_start(cp_sb[:], cp[:])
            cidb_sb = const_pool.tile([BLK, SEQ], F32, tag="cidb")
            nc.sync.dma_start(cidb_sb[:], cidb[:])
            jvec_sb = const_pool.tile([BLK, ncols], F32, tag="jvec")
            nc.sync.dma_start(jvec_sb[:], jvec[:])
            ema = const_pool.tile([BLK, NBLK * DIM], F32, tag="ema")

            # ---- blocked matmul scan over chunk blocks ----
            hprev = None
            for t in range(NBLK):
                xt = xpool.tile([BLK, DIM], F32, tag="xt")
                nc.sync.dma_start(xt[:], x[t * BLK:(t + 1) * BLK, :])
                ps = ps_scan.tile([BLK, DIM], F32, tag="ps")
                for h in range(2):
                    sl = slice(h * 512, (h + 1) * 512)
                    nc.tensor.matmul(
                        ps[:, sl],
                        lhsT=lt_sb[:, t * BLK:(t + 1) * BLK],
                        rhs=xt[:, sl],
                        start=True,
                        stop=(t == 0),
                    )
                if t > 0:
                    for h in range(2):
                        sl = slice(h * 512, (h + 1) * 512)
                        nc.tensor.matmul(
                            ps[:, sl],
                            lhsT=cp_sb[:, t * BLK:(t + 1) * BLK],
                            rhs=hprev[:, sl],
                            start=False,
                            stop=True,
                        )
                nc.vector.tensor_copy(out=ema[:, t * DIM:(t + 1) * DIM], in_=ps[:])
                if t < NBLK - 1:
                    hp = hpool.tile([1, DIM], F32, tag="hp")
                    nc.vector.tensor_copy(out=hp[:], in_=ps[0:1, :])
                    hprev = hp

            # ---- dechunk: one-hot selection matmuls per token block ----
            col = 0
            for tb in range(NTB):
                w = windows[tb]
                sels = []
                for t in w:
                    sel = selpool.tile([BLK, BLK], F32, tag="sel")
                    nc.vector.tensor_scalar(
                        out=sel[:],
                        in0=cidb_sb[:, tb * BLK:(tb + 1) * BLK],
                        scalar1=jvec_sb[:, col:col + 1],
                        scalar2=None,
                        op0=mybir.AluOpType.is_equal,
                    )
                    sels.append((sel, t))
                    col += 1
                po = ps_out.tile([BLK, DIM], F32, tag="po")
                for wi, (sel, t) in enumerate(sels):
                    for h in range(2):
                        sl = slice(h * 512, (h + 1) * 512)
                        nc.tensor.matmul(
                            po[:, sl],
                            lhsT=sel[:],
                            rhs=ema[:, t * DIM + h * 512: t * DIM + (h + 1) * 512],
                            start=(wi == 0),
                            stop=(wi == len(sels) - 1),
                        )
                ob = outpool.tile([BLK, DIM], F32, tag="ob")
                nc.any.tensor_copy(out=ob[:], in_=po[:])
                nc.sync.dma_start(out[tb * BLK:(tb + 1) * BLK, :], ob[:])
    nc.finalize()
    return nc


def kernel(chunk_states, boundary_mask, boundary_prob):
    in_maps, NBLK, windows = _preprocess(chunk_states, boundary_mask, boundary_prob)
    nc = _build_nc(NBLK, windows)
    res = run_bass_kernel_spmd(nc, in_maps, core_ids=list(range(NCORES)))
    return np.stack([res.results[i]["out"] for i in range(NCORES)], axis=0)


# revision 6
# speedup vs baseline: 1.7697x; 1.7697x over previous
"""Trainium2 Bass kernel for nn_DeChunkLayer.

Computation (per batch row):
  1. gate[c]: boundary-sorted clipped probabilities (host, tiny).
  2. EMA linear recurrence over chunks: h_c = (1-g_c) h_{c-1} + g_c x_c,
     computed on-device as a blocked lower-triangular matmul scan:
     per 128-chunk block t:  ema_t = L_t @ X_t + cp_t (x) h_prev
     where L_t[i,j] = g_j * prod_{k=j+1..i}(1-g_k), cp_t[i] = prod_{k<=i}(1-g_k)
     (host-computed in f64 via log-space cumsums).
  3. Dechunk: out[s] = ema[cid[s]] done as one-hot selection matmuls
     per 128-token block; selection matrices built on-device with
     is_equal against the replicated chunk-id row.

Sharding: pure data parallel, one batch row per NeuronCore (B=8, 8 cores).

Only chunk blocks that are actually referenced by tokens are processed
(NBLK = ceil(max_used_chunks/128)), all cores share one SPMD program whose
dechunk windows are the union of the per-core windows.
"""

import math

import numpy as np

import concourse.bass as bass
import concourse.mybir as mybir
from concourse import tile
from concourse.bass_utils import run_bass_kernel_spmd

B, SEQ, MAXC, DIM = 8, 4096, 2048, 1024
BLK = 128
NCORES = 8
NTB = SEQ // BLK  # 32 token blocks
F32 = mybir.dt.float32
F16 = mybir.dt.float16


def _preprocess(chunk_states, boundary_mask, boundary_prob):
    """Host-side index/gate math. Returns (in_maps, NBLK, windows)."""
    chunk_states = np.asarray(chunk_states, dtype=np.float32)
    boundary_mask = np.asarray(boundary_mask)
    boundary_prob = np.asarray(boundary_prob, dtype=np.float32)

    p_full = np.clip(boundary_prob[..., -1], np.float32(1e-4), np.float32(1.0 - 1e-4))
    token_idx = np.arange(SEQ)[None, :] + (~boundary_mask).astype(np.int32) * SEQ
    order = np.argsort(token_idx, axis=1, kind="stable")
    gate = np.take_along_axis(p_full, order[:, :MAXC], axis=1)  # [B, C]

    cid = np.cumsum(boundary_mask.astype(np.int32), axis=1) - 1  # [B, S]
    cid = np.clip(cid, 0, MAXC - 1)
    n_used = int(cid.max()) + 1
    NBLK = max(1, math.ceil(n_used / BLK))
    CU = NBLK * BLK

    # Per-block L^T and cp, in f64 log space (underflow -> 0 is fine).
    g = gate[:, :CU].astype(np.float64)
    a = 1.0 - g
    ls = np.cumsum(np.log(a).reshape(B, NBLK, BLK), axis=2)  # [B, NBLK, BLK]
    ii = np.arange(BLK)[:, None]
    jj = np.arange(BLK)[None, :]
    # L[b,t,i,j] = g[j] * exp(ls[i]-ls[j]) for i>=j
    Lf = np.where(
        ii[None, None] >= jj[None, None],
        np.exp(ls[:, :, :, None] - ls[:, :, None, :])
        * g.reshape(B, NBLK, 1, BLK),
        0.0,
    ).astype(np.float32)  # [B, NBLK, i, j]
    # ema rows are stored partition-REVERSED (chunk i of a block sits on
    # partition 127-i) so the block-carry row (last chunk) is partition 0,
    # which compute engines may legally address ([1,*] APs need base 0).
    Lf = Lf[:, :, ::-1, :]
    # lhsT layout: LT_sb[j, t*BLK+m] = L[t, rev(m), j]
    LT_sb = np.ascontiguousarray(
        Lf.transpose(0, 3, 1, 2).reshape(B, BLK, NBLK * BLK).astype(np.float16)
    )
    cp = np.exp(ls).astype(np.float16)[:, :, ::-1]  # [B, NBLK, BLK] reversed
    cp_sb = np.ascontiguousarray(cp.reshape(B, 1, NBLK * BLK))

    # dechunk union windows per token block
    cidr = cid.reshape(B, NTB, BLK)
    t0 = cidr[:, :, 0] // BLK  # [B, NTB]
    t1 = cidr[:, :, -1] // BLK
    lo = t0.min(axis=0)  # [NTB]
    hi = t1.max(axis=0)
    windows = [list(range(int(lo[tb]), int(hi[tb]) + 1)) for tb in range(NTB)]
    ncols = sum(len(w) for w in windows)
    jvec = np.empty((BLK, ncols), dtype=np.float32)
    col = 0
    for tb in range(NTB):
        for t in windows[tb]:
            jvec[:, col] = t * BLK + (BLK - 1 - np.arange(BLK))
            col += 1

    in_maps = []
    for b in range(B):
        in_maps.append(
            {
                "x": np.ascontiguousarray(chunk_states[b, :CU].astype(np.float16)),
                "lt": LT_sb[b],
                "cp": cp_sb[b],
                "cidb": np.ascontiguousarray(
                    np.broadcast_to(
                        cid[b].astype(np.float16)[None, :], (BLK, SEQ)
                    )
                ),
                "jvec": jvec,
            }
        )
    return in_maps, NBLK, windows


def _build_nc(NBLK, windows):
    ncols = sum(len(w) for w in windows)
    nc = bass.Bass()
    x = nc.dram_tensor("x", [NBLK * BLK, DIM], F16, kind="ExternalInput")
    lt = nc.dram_tensor("lt", [BLK, NBLK * BLK], F16, kind="ExternalInput")
    cp = nc.dram_tensor("cp", [1, NBLK * BLK], F16, kind="ExternalInput")
    cidb = nc.dram_tensor("cidb", [BLK, SEQ], F16, kind="ExternalInput")
    jvec = nc.dram_tensor("jvec", [BLK, ncols], F32, kind="ExternalInput")
    out = nc.dram_tensor("out", [SEQ, DIM], F32, kind="ExternalOutput")

    with tile.TileContext(nc) as tc:
        with (
            tc.tile_pool(name="const", bufs=1) as const_pool,
            tc.tile_pool(name="xp", bufs=4) as xpool,
            tc.tile_pool(name="hp", bufs=2) as hpool,
            tc.tile_pool(name="selp", bufs=8) as selpool,
            tc.tile_pool(name="outp", bufs=4) as outpool,
            tc.tile_pool(name="ps_scan", bufs=2, space="PSUM") as ps_scan,
            tc.tile_pool(name="ps_out", bufs=2, space="PSUM") as ps_out,
        ):
            lt_sb = const_pool.tile([BLK, NBLK * BLK], F16, tag="lt")
            nc.sync.dma_start(lt_sb[:], lt[:])
            cp_sb = const_pool.tile([1, NBLK * BLK], F16, tag="cp")
            nc.sync.dma_start(cp_sb[:], cp[:])
            cidb_sb = const_pool.tile([BLK, SEQ], F16, tag="cidb")
            nc.sync.dma_start(cidb_sb[:], cidb[:])
            jvec_sb = const_pool.tile([BLK, ncols], F32, tag="jvec")
            nc.sync.dma_start(jvec_sb[:], jvec[:])
            ema = const_pool.tile([BLK, NBLK * DIM], F16, tag="ema")

            # ---- blocked matmul scan over chunk blocks ----
            hprev = None
            for t in range(NBLK):
                xt = xpool.tile([BLK, DIM], F16, tag="xt")
                nc.sync.dma_start(xt[:], x[t * BLK:(t + 1) * BLK, :])
                ps = ps_scan.tile([BLK, DIM], F32, tag="ps")
                for h in range(2):
                    sl = slice(h * 512, (h + 1) * 512)
                    nc.tensor.matmul(
                        ps[:, sl],
                        lhsT=lt_sb[:, t * BLK:(t + 1) * BLK],
                        rhs=xt[:, sl],
                        start=True,
                        stop=(t == 0),
                    )
                if t > 0:
                    for h in range(2):
                        sl = slice(h * 512, (h + 1) * 512)
                        nc.tensor.matmul(
                            ps[:, sl],
                            lhsT=cp_sb[:, t * BLK:(t + 1) * BLK],
                            rhs=hprev[:, sl],
                            start=False,
                            stop=True,
                        )
                nc.vector.tensor_copy(out=ema[:, t * DIM:(t + 1) * DIM], in_=ps[:])
                if t < NBLK - 1:
                    hp = hpool.tile([1, DIM], F16, tag="hp")
                    nc.vector.tensor_copy(out=hp[:], in_=ps[0:1, :])
                    hprev = hp

            # ---- dechunk: one-hot selection matmuls per token block ----
            col = 0
            for tb in range(NTB):
                w = windows[tb]
                sels = []
                for t in w:
                    sel = selpool.tile([BLK, BLK], F16, tag="sel")
                    nc.vector.tensor_scalar(
                        out=sel[:],
                        in0=cidb_sb[:, tb * BLK:(tb + 1) * BLK],
                        scalar1=jvec_sb[:, col:col + 1],
                        scalar2=None,
                        op0=mybir.AluOpType.is_equal,
                    )
                    sels.append((sel, t))
                    col += 1
                po = ps_out.tile([BLK, DIM], F32, tag="po")
                for wi, (sel, t) in enumerate(sels):
                    for h in range(2):
                        sl = slice(h * 512, (h + 1) * 512)
                        nc.tensor.matmul(
                            po[:, sl],
                            lhsT=sel[:],
                            rhs=ema[:, t * DIM + h * 512: t * DIM + (h + 1) * 512],
                            start=(wi == 0),
                            stop=(wi == len(sels) - 1),
                        )
                ob = outpool.tile([BLK, DIM], F32, tag="ob")
                nc.any.tensor_copy(out=ob[:], in_=po[:])
                nc.sync.dma_start(out[tb * BLK:(tb + 1) * BLK, :], ob[:])
    nc.finalize()
    return nc


def kernel(chunk_states, boundary_mask, boundary_prob):
    in_maps, NBLK, windows = _preprocess(chunk_states, boundary_mask, boundary_prob)
    nc = _build_nc(NBLK, windows)
    res = run_bass_kernel_spmd(nc, in_maps, core_ids=list(range(NCORES)))
    return np.stack([res.results[i]["out"] for i in range(NCORES)], axis=0)
